# revision 14
# baseline (speedup 1.0000x reference)
"""Trainium2 Bass kernel for a codebook-contrastive head (vq_codebook).

reference computation, for query_features x [B=1024, Q=95, D=1024] f32 and
embedding table weight [114, D] f32 (19 classes x (5 prototypes + 1 bg row)):

    logits [B, Q, 20] = -inf everywhere except
      logits[b, q, c(q)] = max_{k<5} <x[b,q,:], weight[6*c(q)+k, :]>
      logits[b, q, 19]   =           <x[b,q,:], weight[6*c(q)+5, :]>
    with c(q) = q // 5.

Distribution: pure batch data-parallelism over 8 NeuronCores (128 batch rows
per core), embedding table replicated — no collectives.

Per-core device kernel (memory-bound; DMA of the query shard is the
roofline):
  * The host pre-transposes the shard so the contraction dim D sits on SBUF
    partitions (removes all on-chip transposes) and casts to bf16 (halves
    DMA traffic; norm rel err ~2e-3, far under the 2e-2 gate).
  * Queries stream in chunks of 25/20/15/10 (whole classes); each chunk is
    8 per-d-chunk DMAs (~0.8 MB each, ~6 KB contiguous per partition)
    alternating across the two HWDGE rings (SP + ACT), double-buffered.
    Descending chunk sizes keep the final compute+store tail short.
  * Per query: 8 accumulating matmuls, lhsT = X^T d-chunk [128d, 128b]
    (stationary), rhs = the 6 prototype columns of the query's class
    [128d, 6] (moving) -> PSUM [128b, 6].
  * Per class: one vector reduce_max over the 5 prototype columns and one
    copy of the background column into a -inf-memset output tile; chunk
    results are streamed back to DRAM so only the last small store is a tail.
  * Built as bacc.Bacc + nc.compile(): the event-semaphore pass splits
    multi-sem waits (the TPB ISA encodes at most one wait per instruction).
"""

import numpy as np
import ml_dtypes

import concourse.bass as bass
import concourse.bacc as bacc
import concourse.mybir as mybir
import concourse.tile as tile
from concourse.bass_utils import run_bass_kernel_spmd

N_CORES = 8
B, Q, D = 1024, 95, 1024
NCLS = 19              # classes
QPC = 5                # queries per class
PPC = 5                # prototypes per class
KROWS = PPC + 1        # table rows per class (5 protos + 1 background)
NPROTO = NCLS * KROWS  # 114
BLOC = B // N_CORES    # 128 batch rows per core
DK = D // 128          # 8 contraction chunks of 128
SPLITS = (25, 25, 20, 15, 10)  # query chunks (each a whole number of classes)

BF16 = mybir.dt.bfloat16
F32 = mybir.dt.float32

_BUILD_CACHE = None
LAST_RESULT = None    # BassKernelResults of the most recent run (for test.py)
LAST_IN_MAPS = None   # per-core input maps of the most recent run (for test.py)


def _build():
    nc = bacc.Bacc()
    xt = nc.declare_dram_parameter("xt", [DK, 128, Q, BLOC], BF16, isOutput=False)
    wt = nc.declare_dram_parameter("wt", [DK, 128, NPROTO], BF16, isOutput=False)
    out = nc.declare_dram_parameter("out", [BLOC, Q, NCLS + 1], F32, isOutput=True)

    max_split = max(SPLITS)
    dma_engines = ("sync", "scalar")
    with tile.TileContext(nc) as tc:
        with (
            tc.tile_pool(name="wpool", bufs=1) as wpool,
            tc.tile_pool(name="xpool", bufs=2) as xpool,
            tc.tile_pool(name="opool", bufs=1) as opool,
            tc.tile_pool(name="mpool", bufs=2) as mpool,
            tc.tile_pool(name="psum", bufs=8, space="PSUM") as pspool,
        ):
            wt_sb = wpool.tile([128, DK, NPROTO], BF16)
            for k in range(DK):
                nc.sync.dma_start(out=wt_sb[:, k, :], in_=wt[k])

            out_sb = opool.tile([128, Q, NCLS + 1], F32)
            nc.vector.memset(out_sb[:], float("-inf"))

            di = 0
            q0 = 0
            for nq in SPLITS:
                xq = xpool.tile([128, DK, max_split * BLOC], BF16, tag="xq")
                for k in range(DK):
                    eng = getattr(nc, dma_engines[di % len(dma_engines)])
                    eng.dma_start(
                        out=xq[:, k, : nq * BLOC],
                        in_=xt[k].rearrange("p q b -> p (q b)")[
                            :, q0 * BLOC : (q0 + nq) * BLOC
                        ],
                    )
                    di += 1
                for c in range(q0 // QPC, (q0 + nq) // QPC):
                    cq0 = c * QPC
                    pb = pspool.tile([128, QPC, KROWS], F32, tag="pb")
                    for qi in range(QPC):
                        xoff = (cq0 + qi - q0) * BLOC
                        for k in range(DK):
                            nc.tensor.matmul(
                                pb[:, qi, :],
                                xq[:, k, xoff : xoff + BLOC],
                                wt_sb[:, k, KROWS * c : KROWS * c + KROWS],
                                start=(k == 0),
                                stop=(k == DK - 1),
                            )
                    pm = mpool.tile([128, QPC], F32, tag="pm")
                    nc.vector.reduce_max(
                        pm[:], pb[:, :, 0:PPC], axis=mybir.AxisListType.X
                    )
                    nc.vector.tensor_copy(out_sb[:, cq0 : cq0 + QPC, c], pm[:])
                    nc.vector.tensor_copy(
                        out_sb[:, cq0 : cq0 + QPC, NCLS], pb[:, :, PPC]
                    )
                nc.gpsimd.dma_start(
                    out=out[:, q0 : q0 + nq, :], in_=out_sb[:, q0 : q0 + nq, :]
                )
                q0 += nq
    nc.compile()
    return nc


def kernel(query_features, weight, num_classes=19, queries_per_class=5,
           prototypes_per_class=5, **_ignored):
    global _BUILD_CACHE, LAST_RESULT, LAST_IN_MAPS
    assert int(num_classes) == NCLS
    assert int(queries_per_class) == QPC
    assert int(prototypes_per_class) == PPC

    x = np.asarray(query_features, dtype=np.float32)
    w = np.asarray(weight, dtype=np.float32)
    assert x.shape == (B, Q, D) and w.shape == (NPROTO, D)

    # wt[k, p, j] = weight[j, 128k + p], bf16
    wt_np = np.ascontiguousarray(
        w.T.reshape(DK, 128, NPROTO).astype(ml_dtypes.bfloat16)
    )

    in_maps = []
    xb = x.astype(ml_dtypes.bfloat16)
    for i in range(N_CORES):
        xi = xb[i * BLOC : (i + 1) * BLOC]          # [128b, 95q, 1024d]
        # xt[k, p, q, b] = x[b, q, 128k + p]
        xt_np = np.ascontiguousarray(
            xi.transpose(2, 1, 0).reshape(DK, 128, Q, BLOC)
        )
        in_maps.append({"xt": xt_np, "wt": wt_np})

    if _BUILD_CACHE is None:
        _BUILD_CACHE = _build()
    nc = _BUILD_CACHE

    res = run_bass_kernel_spmd(nc, in_maps, core_ids=list(range(N_CORES)))
    LAST_RESULT = res
    LAST_IN_MAPS = in_maps
    out = np.concatenate(
        [np.asarray(res.results[i]["out"]) for i in range(N_CORES)], axis=0
    )
    return out.astype(np.float32)


# revision 15
# speedup vs baseline: 1.0371x; 1.0371x over previous
"""Trainium2 Bass kernel for a codebook-contrastive head (vq_codebook).

reference computation, for query_features x [B=1024, Q=95, D=1024] f32 and
embedding table weight [114, D] f32 (19 classes x (5 prototypes + 1 bg row)):

    logits [B, Q, 20] = -inf everywhere except
      logits[b, q, c(q)] = max_{k<5} <x[b,q,:], weight[6*c(q)+k, :]>
      logits[b, q, 19]   =           <x[b,q,:], weight[6*c(q)+5, :]>
    with c(q) = q // 5.

Distribution: pure batch data-parallelism over 8 NeuronCores (128 batch rows
per core), embedding table replicated — no collectives.

Per-core device kernel (memory-bound; DMA of the query shard is the
roofline):
  * The host pre-transposes the shard so the contraction dim D sits on SBUF
    partitions (removes all on-chip transposes) and casts to bf16 (halves
    DMA traffic; norm rel err ~2e-3, far under the 2e-2 gate).
  * Queries stream in chunks of 25/20/15/10 (whole classes); each chunk is
    8 per-d-chunk DMAs (~0.8 MB each, ~6 KB contiguous per partition)
    alternating across the two HWDGE rings (SP + ACT), double-buffered.
    Descending chunk sizes keep the final compute+store tail short.
  * Per query: 8 accumulating matmuls, lhsT = X^T d-chunk [128d, 128b]
    (stationary), rhs = the 6 prototype columns of the query's class
    [128d, 6] (moving) -> PSUM [128b, 6].
  * Per class: one vector reduce_max over the 5 prototype columns and one
    copy of the background column into a -inf-memset output tile; chunk
    results are streamed back to DRAM so only the last small store is a tail.
  * Built as bacc.Bacc + nc.compile(): the event-semaphore pass splits
    multi-sem waits (the TPB ISA encodes at most one wait per instruction).
"""

import numpy as np
import ml_dtypes

import concourse.bacc as bacc
import concourse.mybir as mybir
import concourse.tile as tile
from concourse.bass_utils import run_bass_kernel_spmd

N_CORES = 8
B, Q, D = 1024, 95, 1024
NCLS = 19              # classes
QPC = 5                # queries per class
PPC = 5                # prototypes per class
KROWS = PPC + 1        # table rows per class (5 protos + 1 background)
NPROTO = NCLS * KROWS  # 114
BLOC = B // N_CORES    # 128 batch rows per core
DK = D // 128          # 8 contraction chunks of 128
SPLITS = (25, 25, 20, 15, 10)  # query chunks (each a whole number of classes)

BF16 = mybir.dt.bfloat16
F32 = mybir.dt.float32

_BUILD_CACHE = None
LAST_RESULT = None    # BassKernelResults of the most recent run (for test.py)
LAST_IN_MAPS = None   # per-core input maps of the most recent run (for test.py)


def _build():
    nc = bacc.Bacc()
    xt = nc.declare_dram_parameter("xt", [DK, 128, Q, BLOC], BF16, isOutput=False)
    wt = nc.declare_dram_parameter("wt", [DK, 128, NPROTO], BF16, isOutput=False)
    out = nc.declare_dram_parameter("out", [BLOC, Q, NCLS + 1], F32, isOutput=True)

    max_split = max(SPLITS)
    dma_engines = ("sync", "scalar")
    with tile.TileContext(nc) as tc:
        with (
            tc.tile_pool(name="wpool", bufs=1) as wpool,
            tc.tile_pool(name="xpool", bufs=2) as xpool,
            tc.tile_pool(name="opool", bufs=1) as opool,
            tc.tile_pool(name="mpool", bufs=2) as mpool,
            tc.tile_pool(name="psum", bufs=8, space="PSUM") as pspool,
        ):
            wt_sb = wpool.tile([128, DK, NPROTO], BF16)
            for k in range(DK):
                nc.sync.dma_start(out=wt_sb[:, k, :], in_=wt[k])

            out_sb = opool.tile([128, Q, NCLS + 1], F32)
            nc.vector.memset(out_sb[:], float("-inf"))

            di = 0
            q0 = 0
            for nq in SPLITS:
                xq = xpool.tile([128, DK, max_split * BLOC], BF16, tag="xq")
                for k in range(DK):
                    eng = getattr(nc, dma_engines[di % len(dma_engines)])
                    eng.dma_start(
                        out=xq[:, k, : nq * BLOC],
                        in_=xt[k].rearrange("p q b -> p (q b)")[
                            :, q0 * BLOC : (q0 + nq) * BLOC
                        ],
                    )
                    di += 1
                for c in range(q0 // QPC, (q0 + nq) // QPC):
                    cq0 = c * QPC
                    pb = pspool.tile([128, QPC, KROWS], F32, tag="pb")
                    for qi in range(QPC):
                        xoff = (cq0 + qi - q0) * BLOC
                        for k in range(DK):
                            nc.tensor.matmul(
                                pb[:, qi, :],
                                xq[:, k, xoff : xoff + BLOC],
                                wt_sb[:, k, KROWS * c : KROWS * c + KROWS],
                                start=(k == 0),
                                stop=(k == DK - 1),
                            )
                    pm = mpool.tile([128, QPC], F32, tag="pm")
                    nc.vector.reduce_max(
                        pm[:], pb[:, :, 0:PPC], axis=mybir.AxisListType.X
                    )
                    nc.vector.tensor_copy(out_sb[:, cq0 : cq0 + QPC, c], pm[:])
                    nc.vector.tensor_copy(
                        out_sb[:, cq0 : cq0 + QPC, NCLS], pb[:, :, PPC]
                    )
                nc.gpsimd.dma_start(
                    out=out[:, q0 : q0 + nq, :], in_=out_sb[:, q0 : q0 + nq, :]
                )
                q0 += nq
    nc.compile()
    return nc


def kernel(query_features, weight, num_classes=19, queries_per_class=5,
           prototypes_per_class=5, **_ignored):
    global _BUILD_CACHE, LAST_RESULT, LAST_IN_MAPS
    assert int(num_classes) == NCLS
    assert int(queries_per_class) == QPC
    assert int(prototypes_per_class) == PPC

    x = np.asarray(query_features, dtype=np.float32)
    w = np.asarray(weight, dtype=np.float32)
    assert x.shape == (B, Q, D) and w.shape == (NPROTO, D)

    # wt[k, p, j] = weight[j, 128k + p], bf16
    wt_np = np.ascontiguousarray(
        w.T.reshape(DK, 128, NPROTO).astype(ml_dtypes.bfloat16)
    )

    in_maps = []
    xb = x.astype(ml_dtypes.bfloat16)
    for i in range(N_CORES):
        xi = xb[i * BLOC : (i + 1) * BLOC]          # [128b, 95q, 1024d]
        # xt[k, p, q, b] = x[b, q, 128k + p]
        xt_np = np.ascontiguousarray(
            xi.transpose(2, 1, 0).reshape(DK, 128, Q, BLOC)
        )
        in_maps.append({"xt": xt_np, "wt": wt_np})

    if _BUILD_CACHE is None:
        _BUILD_CACHE = _build()
    nc = _BUILD_CACHE

    res = run_bass_kernel_spmd(nc, in_maps, core_ids=list(range(N_CORES)))
    LAST_RESULT = res
    LAST_IN_MAPS = in_maps
    out = np.concatenate(
        [np.asarray(res.results[i]["out"]) for i in range(N_CORES)], axis=0
    )
    return out.astype(np.float32)
